# revision 16
# baseline (speedup 1.0000x reference)
"""Sliding-window (left-W, strictly causal) dot attention + sigmoid output head.

Reference computation (per batch b, step t):
    window  = padded positions t..t+W-1 of h = pad(x, W zeros at front)
    scores  = <x_t, h_s>, softmax over the W window slots (padding included)
    c_t     = sum_w alpha_w * h_w
    out     = sigmoid(concat([c_t, x_t]) @ W_c)

Shapes: x [4, 2048, 512] f32, W_c [1024, 512] f32, W = 64.

Sharding: 8 shards = (batch b, sequence half) pairs. Each core gets its
sequence half plus the W-row halo on the left (zeros for the first half),
so no inter-core communication is needed. W_c is replicated.

Per-core formulation (L = 1024 queries, D = 512, W = 64):
    out = sigmoid(alpha @ G + x @ Wc_bot),  G = hp @ Wc_top
associativity moves the context-vector contraction onto the precomputed
G [P, 512], so the raw window rows (hp natural layout), the context
vector, and its transposes are never materialized. Only the transposed
activations hpT [512, P] are needed on chip.

Fully unrolled over 8 query blocks of 128:
  - scores S[r, c] = <q_r, key_{128i+c}> over a 256-wide key span via 4
    fp32r matmuls (free dim 256 hits fp32r full rate; fp32 runs 1/4).
    Band mask (0 <= c - r < 64) additive -1e30.
  - softmax along free dim: DVE mask-add, reduce_max(negate), ACT
    Exp(bias=-max, accum_out=rowsum), reciprocal, normalize on DVE.
  - alpha^T via 2 PE transposes (one PSUM tile), copy to SBUF.
  - head: y_psum accumulates 2 matmuls alpha^T x G-rows + 4 matmuls
    x^T x Wc_bot, all fp32r N=512.
  - sigmoid as 0.5*tanh(0.5 z) + 0.5: exp and tanh live in the same
    activation-function table set, so the ACT engine never reloads
    tables (a dedicated Sigmoid would force one reload per block).
"""

import numpy as np

import concourse.bass as bass
import concourse.mybir as mybir
import concourse.tile as tile
from concourse import bacc
from concourse.bass_utils import run_bass_kernel_spmd

B = 4
S = 2048
D = 512
W = 64
O = 512
NCORES = 8
L = S // 2          # queries per core
P = W + L + W       # padded positions per core: halo + queries + tail pad
NBLK = L // 128     # query blocks per core
F32 = mybir.dt.float32
F32R = mybir.dt.float32r
NEG = -1.0e30
AF = mybir.ActivationFunctionType


def build_kernel(n_repeats: int = 1, affine_dve: bool = True):
    """Build + bacc-compile the per-core Bass module.

    n_repeats re-emits the whole compute body (same inputs, same output
    tiles) for wall-clock difference timing; results are identical.
    """
    nc = bacc.Bacc()
    hpT_d = nc.declare_dram_parameter("hpT", [D, P], F32R, isOutput=False)
    wc_d = nc.declare_dram_parameter("wc", [2 * D, O], F32R, isOutput=False)
    out_d = nc.declare_dram_parameter("out", [L, O], F32, isOutput=True)

    with tile.TileContext(nc) as tc:
        with (
            tc.tile_pool(name="resident", bufs=1) as res,
            tc.tile_pool(name="work", bufs=2) as wk,
            tc.tile_pool(name="stats", bufs=3) as st,
            tc.tile_pool(name="ps_s", bufs=4, space="PSUM") as ps_s,
            tc.tile_pool(name="ps_et", bufs=2, space="PSUM") as ps_et,
            tc.tile_pool(name="ps_y", bufs=2, space="PSUM") as ps_y,
        ):
            # ---- resident inputs ----
            # hpT arrives in column halves: the first half (positions < P/2)
            # unblocks the first four score blocks and all early G tiles
            # while wc and the second half stream in behind it.
            hpT_sb = []
            CUT1, CUT2 = 384, 704   # scores 0-1 need <384; 0-3 need <704
            for dc in range(D // 128):
                t = res.tile([128, P], F32R, tag=f"hpT{dc}", name=f"hpT{dc}")
                nc.sync.dma_start(
                    t[:, 0:CUT1], hpT_d[128 * dc : 128 * (dc + 1), 0:CUT1])
                hpT_sb.append(t)
            wc_sb = []
            for fc in range(2 * D // 128):
                t = res.tile([128, O], F32R, tag=f"wc{fc}", name=f"wc{fc}")
                nc.sync.dma_start(t[:], wc_d[128 * fc : 128 * (fc + 1), :])
                wc_sb.append(t)
                if fc == 3:
                    for dc in range(D // 128):
                        nc.sync.dma_start(
                            hpT_sb[dc][:, CUT1:CUT2],
                            hpT_d[128 * dc : 128 * (dc + 1), CUT1:CUT2])
            for dc in range(D // 128):
                nc.sync.dma_start(
                    hpT_sb[dc][:, CUT2:P],
                    hpT_d[128 * dc : 128 * (dc + 1), CUT2:P])

            # band mask [128, 192]: 0 where 0 <= c - r <= W-1 else -1e30.
            # Score tiles are 256 wide (fp32r full-rate needs N >= 256) but
            # only columns 0..190 can be in-band, so softmax ops run on 192.
            mask = res.tile([128, 192], F32, tag="mask")
            nc.gpsimd.memset(mask[:], 0.0)
            nc.gpsimd.affine_select(
                out=mask[:], in_=mask[:],
                compare_op=mybir.AluOpType.is_ge, fill=NEG,
                base=0, pattern=[[1, 192]], channel_multiplier=-1,
            )  # keep where c - r >= 0
            nc.gpsimd.affine_select(
                out=mask[:], in_=mask[:],
                compare_op=mybir.AluOpType.is_ge, fill=NEG,
                base=W - 1, pattern=[[-1, 192]], channel_multiplier=1,
            )  # keep where r - c + (W-1) >= 0
            ident = res.tile([128, 128], F32, tag="ident")
            nc.gpsimd.memset(ident[:], 0.0)
            nc.gpsimd.affine_select(
                out=ident[:], in_=ident[:],
                compare_op=mybir.AluOpType.not_equal, fill=1.0,
                base=0, pattern=[[-1, 128]], channel_multiplier=1,
            )
            ident_r = ident[:]  # plain f32 transpose operand

            g_sb = [res.tile([128, O], F32R, tag=f"G{pt}", name=f"G{pt}")
                    for pt in range(P // 128)]

            def emit_g(pt):
                # G[pt] = hp[128pt:128pt+128] @ Wc_top
                g_ps = ps_y.tile([128, O], F32, tag="Y", name="g_ps")
                for dc in range(D // 128):
                    nc.tensor.matmul(
                        g_ps[:],
                        lhsT=hpT_sb[dc][:, 128 * pt : 128 * (pt + 1)],
                        rhs=wc_sb[dc][:],
                        start=(dc == 0), stop=(dc == D // 128 - 1),
                    )
                nc.vector.tensor_copy(g_sb[pt][:], g_ps[:])

            def emit_scores(i):
                # S[r, c] = <q_r, key_{128i+c}>, c in [0, 256)
                qs = W + 128 * i
                ks = 128 * i
                s_ps = ps_s.tile([128, 256], F32, tag="S", name="s_ps")
                for dc in range(D // 128):
                    nc.tensor.matmul(
                        s_ps[:],
                        lhsT=hpT_sb[dc][:, qs : qs + 128],
                        rhs=hpT_sb[dc][:, ks : ks + 256],
                        start=(dc == 0), stop=(dc == D // 128 - 1),
                    )
                return s_ps

            def emit_tail(i, s_ps):
                qs = W + 128 * i
                # masked scores + negated row max (tensor_tensor_reduce would
                # fuse these but crashes the runtime on this compiler)
                nsm = wk.tile([128, 192], F32, tag="nsm", name="nsm")
                nmax = st.tile([128, 1], F32, tag="nmax", name="nmax")
                nc.vector.tensor_add(nsm[:], s_ps[:, 0:192], mask[:])
                nc.vector.tensor_reduce(
                    out=nmax[:], in_=nsm[:], op=mybir.AluOpType.max,
                    axis=mybir.AxisListType.X, negate=True,
                )
                # E = exp(S + mask - max), rowsum into denom
                ex = wk.tile([128, 192], F32, tag="E", name="ex")
                denom = st.tile([128, 1], F32, tag="den", name="denom")
                nc.scalar.activation(
                    ex[:], nsm[:], AF.Exp,
                    bias=nmax[:], scale=1.0, accum_out=denom[:],
                )
                recip = st.tile([128, 1], F32, tag="rec", name="recip")
                nc.vector.reciprocal(recip[:], denom[:])
                en = wk.tile([128, 192], F32, tag="En", name="en")
                nc.vector.tensor_scalar_mul(en[:], ex[:], recip[:])

                # alpha^T via PE transposes into one PSUM tile:
                #   [:, 0:128]      <- keys 0..127 x 128 queries
                #   [0:64, 128:256] <- keys 128..191 x 128 queries
                et_ps = ps_et.tile([128, 256], F32, tag="ETp", name="et_ps")
                nc.tensor.transpose(
                    et_ps[:, 0:128],
                    en[:, 0:128], ident_r)
                nc.tensor.transpose(
                    et_ps[0:64, 128:256],
                    en[:, 128:192], ident_r)
                et = wk.tile([128, 256], F32R, tag="ET", name="et")
                nc.vector.tensor_copy(et[:, 0:128], et_ps[:, 0:128])
                nc.vector.tensor_copy(et[0:64, 128:256], et_ps[0:64, 128:256])

                # head: y = alpha @ G[span] + x @ Wc_bot (6 fp32r matmuls)
                y_ps = ps_y.tile([128, O], F32, tag="Y", name="y_ps")
                nc.tensor.matmul(
                    y_ps[:], lhsT=et[:, 0:128],
                    rhs=g_sb[i][:], start=True, stop=False)
                nc.tensor.matmul(
                    y_ps[:], lhsT=et[0:64, 128:256],
                    rhs=g_sb[i + 1][0:64, :],
                    start=False, stop=False)
                for fc in range(4):
                    nc.tensor.matmul(
                        y_ps[:],
                        lhsT=hpT_sb[fc][:, qs : qs + 128],
                        rhs=wc_sb[4 + fc][:],
                        start=False, stop=(fc == 3),
                    )
                # sigmoid(z) = 0.5 * tanh(0.5 z) + 0.5; tanh shares the Exp
                # ACT table set (no per-block reloads), the affine runs on
                # the otherwise-idle GPSIMD engine.
                # split into half-tiles so tanh/affine/store pipeline and the
                # final block's drain chain is short
                th = wk.tile([128, O], F32, tag="Th", name="th")
                y_sb = wk.tile([128, O], F32, tag="Ysb", name="y_sb")
                eng = nc.vector if affine_dve else nc.gpsimd
                for hh in range(2):
                    sl = slice(hh * (O // 2), (hh + 1) * (O // 2))
                    nc.scalar.activation(th[:, sl], y_ps[:, sl], AF.Tanh,
                                         scale=0.5)
                    eng.tensor_scalar(
                        out=y_sb[:, sl], in0=th[:, sl], scalar1=0.5,
                        scalar2=0.5,
                        op0=mybir.AluOpType.mult, op1=mybir.AluOpType.add,
                    )
                    nc.sync.dma_start(
                        out_d[128 * i : 128 * (i + 1), sl], y_sb[:, sl])

            # Software pipeline: scores run PRE blocks ahead of tails so the
            # in-order PE queue never stalls on a block's softmax; G tiles
            # are staggered between blocks (G[i], G[i+1] ready before
            # tail(i)).
            PRE = 2
            for _ in range(n_repeats):
                sq = {}
                sq[0] = emit_scores(0)
                sq[1] = emit_scores(1)
                emit_g(0)
                emit_g(1)
                for i in range(PRE, NBLK):
                    sq[i] = emit_scores(i)
                    emit_g(i)
                    emit_tail(i - PRE, sq.pop(i - PRE))
                emit_g(NBLK)
                for j in range(NBLK - PRE, NBLK):
                    emit_tail(j, sq.pop(j))

    nc.compile()
    return nc


def shard_inputs(x: np.ndarray, W_c: np.ndarray) -> list[dict[str, np.ndarray]]:
    """Per-core inputs: hp = [left halo (W) | 1024 positions | zero tail],
    shipped transposed as hpT [D, P]."""
    x = np.ascontiguousarray(np.asarray(x, dtype=np.float32))
    W_c = np.ascontiguousarray(np.asarray(W_c, dtype=np.float32))
    in_maps = []
    for k in range(NCORES):
        b, h = divmod(k, 2)
        hp = np.zeros((P, D), np.float32)
        if h == 0:
            hp[W : W + L] = x[b, 0:L]
        else:
            hp[0 : W + L] = x[b, L - W : 2 * L]
        hpT = np.ascontiguousarray(hp.T)
        in_maps.append({"hpT": hpT, "wc": W_c})
    return in_maps


_NC_CACHE: dict[int, object] = {}


def get_nc(n_repeats: int = 1):
    if n_repeats not in _NC_CACHE:
        _NC_CACHE[n_repeats] = build_kernel(n_repeats)
    return _NC_CACHE[n_repeats]


def kernel(x: np.ndarray, W_c: np.ndarray) -> np.ndarray:
    nc = get_nc()
    in_maps = shard_inputs(x, W_c)
    res = run_bass_kernel_spmd(nc, in_maps, list(range(NCORES)))
    out = np.empty((B, S, O), np.float32)
    for k in range(NCORES):
        b, h = divmod(k, 2)
        out[b, h * L : (h + 1) * L] = res.results[k]["out"]
    return out


# revision 20
# speedup vs baseline: 760.1818x; 760.1818x over previous
"""Sliding-window (left-W, strictly causal) dot attention + sigmoid output head.

Reference computation (per batch b, step t):
    window  = padded positions t..t+W-1 of h = pad(x, W zeros at front)
    scores  = <x_t, h_s>, softmax over the W window slots (padding included)
    c_t     = sum_w alpha_w * h_w
    out     = sigmoid(concat([c_t, x_t]) @ W_c)

Shapes: x [4, 2048, 512] f32, W_c [1024, 512] f32, W = 64.

Sharding: 8 shards = (batch b, sequence half) pairs. Each core gets its
sequence half plus the W-row halo on the left (zeros for the first half),
so no inter-core communication is needed. W_c is replicated.

Per-core formulation (L = 1024 queries, D = 512, W = 64):
    out = sigmoid(alpha @ G + x @ Wc_bot),  G = hp @ Wc_top
associativity moves the context-vector contraction onto the precomputed
G [P, 512], so the raw window rows (hp natural layout), the context
vector, and its transposes are never materialized. Only the transposed
activations hpT [512, P] are needed on chip.

Fully unrolled over 8 query blocks of 128:
  - scores S[r, c] = <q_r, key_{128i+c}> over a 256-wide key span via 4
    fp32r matmuls (free dim 256 hits fp32r full rate; fp32 runs 1/4).
    Band mask (0 <= c - r < 64) additive -1e30.
  - softmax along free dim: DVE mask-add, reduce_max(negate), ACT
    Exp(bias=-max, accum_out=rowsum), reciprocal, normalize on DVE.
  - alpha^T via 2 PE transposes (one PSUM tile), copy to SBUF.
  - head: y_psum accumulates 2 matmuls alpha^T x G-rows + 4 matmuls
    x^T x Wc_bot, all fp32r N=512.
  - sigmoid as 0.5*tanh(0.5 z) + 0.5: exp and tanh live in the same
    activation-function table set, so the ACT engine never reloads
    tables (a dedicated Sigmoid would force one reload per block).
"""

import numpy as np

import concourse.bass as bass
import concourse.mybir as mybir
import concourse.tile as tile
from concourse import bacc
from concourse.bass_utils import run_bass_kernel_spmd

B = 4
S = 2048
D = 512
W = 64
O = 512
NCORES = 8
L = S // 2          # queries per core
P = W + L + W       # padded positions per core: halo + queries + tail pad
NBLK = L // 128     # query blocks per core
F32 = mybir.dt.float32
F32R = mybir.dt.float32r
NEG = -1.0e30
AF = mybir.ActivationFunctionType


def build_kernel(n_repeats: int = 1, affine_dve: bool = True, pre: int = 2):
    """Build + bacc-compile the per-core Bass module.

    n_repeats re-emits the whole compute body (same inputs, same output
    tiles) for wall-clock difference timing; results are identical.
    """
    nc = bacc.Bacc()
    hpT_d = nc.declare_dram_parameter("hpT", [D, P], F32R, isOutput=False)
    wc_d = nc.declare_dram_parameter("wc", [2 * D, O], F32R, isOutput=False)
    out_d = nc.declare_dram_parameter("out", [L, O], F32, isOutput=True)

    with tile.TileContext(nc) as tc:
        with (
            tc.tile_pool(name="resident", bufs=1) as res,
            tc.tile_pool(name="work", bufs=2) as wk,
            tc.tile_pool(name="stats", bufs=3) as st,
            tc.tile_pool(name="ps_s", bufs=4, space="PSUM") as ps_s,
            tc.tile_pool(name="ps_et", bufs=2, space="PSUM") as ps_et,
            tc.tile_pool(name="ps_y", bufs=2, space="PSUM") as ps_y,
        ):
            # ---- resident inputs ----
            # hpT arrives in column halves: the first half (positions < P/2)
            # unblocks the first four score blocks and all early G tiles
            # while wc and the second half stream in behind it.
            hpT_sb = []
            CUT1, CUT2 = 384, 704   # scores 0-1 need <384; 0-3 need <704
            for dc in range(D // 128):
                t = res.tile([128, P], F32R, tag=f"hpT{dc}", name=f"hpT{dc}")
                nc.sync.dma_start(
                    t[:, 0:CUT1], hpT_d[128 * dc : 128 * (dc + 1), 0:CUT1])
                hpT_sb.append(t)
            wc_sb = []
            for fc in range(2 * D // 128):
                t = res.tile([128, O], F32R, tag=f"wc{fc}", name=f"wc{fc}")
                nc.sync.dma_start(t[:], wc_d[128 * fc : 128 * (fc + 1), :])
                wc_sb.append(t)
                if fc == 3:
                    for dc in range(D // 128):
                        nc.sync.dma_start(
                            hpT_sb[dc][:, CUT1:CUT2],
                            hpT_d[128 * dc : 128 * (dc + 1), CUT1:CUT2])
            for dc in range(D // 128):
                nc.sync.dma_start(
                    hpT_sb[dc][:, CUT2:P],
                    hpT_d[128 * dc : 128 * (dc + 1), CUT2:P])

            # band mask [128, 192]: 0 where 0 <= c - r <= W-1 else -1e30.
            # Score tiles are 256 wide (fp32r full-rate needs N >= 256) but
            # only columns 0..190 can be in-band, so softmax ops run on 192.
            mask = res.tile([128, 192], F32, tag="mask")
            nc.gpsimd.memset(mask[:], 0.0)
            nc.gpsimd.affine_select(
                out=mask[:], in_=mask[:],
                compare_op=mybir.AluOpType.is_ge, fill=NEG,
                base=0, pattern=[[1, 192]], channel_multiplier=-1,
            )  # keep where c - r >= 0
            nc.gpsimd.affine_select(
                out=mask[:], in_=mask[:],
                compare_op=mybir.AluOpType.is_ge, fill=NEG,
                base=W - 1, pattern=[[-1, 192]], channel_multiplier=1,
            )  # keep where r - c + (W-1) >= 0
            ident = res.tile([128, 128], F32, tag="ident")
            nc.gpsimd.memset(ident[:], 0.0)
            nc.gpsimd.affine_select(
                out=ident[:], in_=ident[:],
                compare_op=mybir.AluOpType.not_equal, fill=1.0,
                base=0, pattern=[[-1, 128]], channel_multiplier=1,
            )
            ident_r = ident[:]  # plain f32 transpose operand

            g_sb = [res.tile([128, O], F32R, tag=f"G{pt}", name=f"G{pt}")
                    for pt in range(P // 128)]

            def emit_g(pt):
                # G[pt] = hp[128pt:128pt+128] @ Wc_top
                g_ps = ps_y.tile([128, O], F32, tag="Y", name="g_ps")
                for dc in range(D // 128):
                    nc.tensor.matmul(
                        g_ps[:],
                        lhsT=hpT_sb[dc][:, 128 * pt : 128 * (pt + 1)],
                        rhs=wc_sb[dc][:],
                        start=(dc == 0), stop=(dc == D // 128 - 1),
                    )
                nc.vector.tensor_copy(g_sb[pt][:], g_ps[:])

            def emit_scores(i):
                # S[r, c] = <q_r, key_{128i+c}>, c in [0, 256)
                qs = W + 128 * i
                ks = 128 * i
                s_ps = ps_s.tile([128, 256], F32, tag="S", name="s_ps")
                for dc in range(D // 128):
                    nc.tensor.matmul(
                        s_ps[:],
                        lhsT=hpT_sb[dc][:, qs : qs + 128],
                        rhs=hpT_sb[dc][:, ks : ks + 256],
                        start=(dc == 0), stop=(dc == D // 128 - 1),
                    )
                return s_ps

            def emit_tail(i, s_ps):
                qs = W + 128 * i
                # masked scores + negated row max (tensor_tensor_reduce would
                # fuse these but crashes the runtime on this compiler)
                nsm = wk.tile([128, 192], F32, tag="nsm", name="nsm")
                nmax = st.tile([128, 1], F32, tag="nmax", name="nmax")
                nc.vector.tensor_add(nsm[:], s_ps[:, 0:192], mask[:])
                nc.vector.tensor_reduce(
                    out=nmax[:], in_=nsm[:], op=mybir.AluOpType.max,
                    axis=mybir.AxisListType.X, negate=True,
                )
                # E = exp(S + mask - max), rowsum into denom
                ex = wk.tile([128, 192], F32, tag="E", name="ex")
                denom = st.tile([128, 1], F32, tag="den", name="denom")
                nc.scalar.activation(
                    ex[:], nsm[:], AF.Exp,
                    bias=nmax[:], scale=1.0, accum_out=denom[:],
                )
                recip = st.tile([128, 1], F32, tag="rec", name="recip")
                nc.vector.reciprocal(recip[:], denom[:])
                en = wk.tile([128, 192], F32, tag="En", name="en")
                nc.vector.tensor_scalar_mul(en[:], ex[:], recip[:])

                # alpha^T via PE transposes into one PSUM tile:
                #   [:, 0:128]      <- keys 0..127 x 128 queries
                #   [0:64, 128:256] <- keys 128..191 x 128 queries
                et_ps = ps_et.tile([128, 256], F32, tag="ETp", name="et_ps")
                nc.tensor.transpose(
                    et_ps[:, 0:128],
                    en[:, 0:128], ident_r)
                nc.tensor.transpose(
                    et_ps[0:64, 128:256],
                    en[:, 128:192], ident_r)
                et = wk.tile([128, 256], F32R, tag="ET", name="et")
                nc.vector.tensor_copy(et[:, 0:128], et_ps[:, 0:128])
                nc.vector.tensor_copy(et[0:64, 128:256],
                                      et_ps[0:64, 128:256])

                # head: y = alpha @ G[span] + x @ Wc_bot (6 fp32r matmuls)
                y_ps = ps_y.tile([128, O], F32, tag="Y", name="y_ps")
                nc.tensor.matmul(
                    y_ps[:], lhsT=et[:, 0:128],
                    rhs=g_sb[i][:], start=True, stop=False)
                nc.tensor.matmul(
                    y_ps[:], lhsT=et[0:64, 128:256],
                    rhs=g_sb[i + 1][0:64, :],
                    start=False, stop=False)
                for fc in range(4):
                    nc.tensor.matmul(
                        y_ps[:],
                        lhsT=hpT_sb[fc][:, qs : qs + 128],
                        rhs=wc_sb[4 + fc][:],
                        start=False, stop=(fc == 3),
                    )
                # sigmoid(z) = 0.5 * tanh(0.5 z) + 0.5; tanh shares the Exp
                # ACT table set (no per-block reloads), the affine runs on
                # the otherwise-idle GPSIMD engine.
                # split into half-tiles so tanh/affine/store pipeline and the
                # final block's drain chain is short
                th = wk.tile([128, O], F32, tag="Th", name="th")
                y_sb = wk.tile([128, O], F32, tag="Ysb", name="y_sb")
                eng = nc.vector if affine_dve else nc.gpsimd
                for hh in range(2):
                    sl = slice(hh * (O // 2), (hh + 1) * (O // 2))
                    nc.scalar.activation(th[:, sl], y_ps[:, sl], AF.Tanh,
                                         scale=0.5)
                    eng.tensor_scalar(
                        out=y_sb[:, sl], in0=th[:, sl], scalar1=0.5,
                        scalar2=0.5,
                        op0=mybir.AluOpType.mult, op1=mybir.AluOpType.add,
                    )
                    nc.sync.dma_start(
                        out_d[128 * i : 128 * (i + 1), sl], y_sb[:, sl])

            # Software pipeline: scores run PRE blocks ahead of tails so the
            # in-order PE queue never stalls on a block's softmax; G tiles
            # are staggered between blocks (G[i], G[i+1] ready before
            # tail(i)).
            PRE = pre
            for _ in range(n_repeats):
                sq = {}
                g_done = 0

                def need_g(upto):
                    nonlocal g_done
                    while g_done <= upto:
                        emit_g(g_done)
                        g_done += 1

                for j in range(PRE):
                    sq[j] = emit_scores(j)
                need_g(1)
                for i in range(PRE, NBLK):
                    sq[i] = emit_scores(i)
                    need_g(i - PRE + 2)
                    emit_tail(i - PRE, sq.pop(i - PRE))
                need_g(P // 128 - 1)
                for j in range(NBLK - PRE, NBLK):
                    emit_tail(j, sq.pop(j))

    nc.compile()
    return nc


def shard_inputs(x: np.ndarray, W_c: np.ndarray) -> list[dict[str, np.ndarray]]:
    """Per-core inputs: hp = [left halo (W) | 1024 positions | zero tail],
    shipped transposed as hpT [D, P]."""
    x = np.ascontiguousarray(np.asarray(x, dtype=np.float32))
    W_c = np.ascontiguousarray(np.asarray(W_c, dtype=np.float32))
    in_maps = []
    for k in range(NCORES):
        b, h = divmod(k, 2)
        hp = np.zeros((P, D), np.float32)
        if h == 0:
            hp[W : W + L] = x[b, 0:L]
        else:
            hp[0 : W + L] = x[b, L - W : 2 * L]
        hpT = np.ascontiguousarray(hp.T)
        in_maps.append({"hpT": hpT, "wc": W_c})
    return in_maps


_NC_CACHE: dict[int, object] = {}


def get_nc(n_repeats: int = 1):
    if n_repeats not in _NC_CACHE:
        _NC_CACHE[n_repeats] = build_kernel(n_repeats)
    return _NC_CACHE[n_repeats]


def kernel(x: np.ndarray, W_c: np.ndarray) -> np.ndarray:
    nc = get_nc()
    in_maps = shard_inputs(x, W_c)
    res = run_bass_kernel_spmd(nc, in_maps, list(range(NCORES)))
    out = np.empty((B, S, O), np.float32)
    for k in range(NCORES):
        b, h = divmod(k, 2)
        out[b, h * L : (h + 1) * L] = res.results[k]["out"]
    return out
